# revision 1
# baseline (speedup 1.0000x reference)
"""AFGNN layer (6-hop sparse message passing + softmax mix + dense proj) on
8 TRN2 NeuronCores — v3.

vs v1 (the 1.97ms baseline):
  - P one-hot build is ONE fused scalar_tensor_tensor per 128-edge block:
    P_b = (iota == slot_b[partition]) * sval_b[partition]  (per-partition
    scalars), instead of two windowed tensor_tensor passes -> DVE drops
    from ~1.4ms to ~0.6ms and no longer stalls the gather pipeline.
  - Self-term mix0*x folded in the epilogue via scalar_tensor_tensor from
    an SBUF f32 repT accumulator (no mix0 identity matmul).
  - Outputs staged per 128-row tile as in v1.
Gather stays SWDGE dma_gather (4 queues, 1024-descriptor calls,
chunk-major int16 indices) — measured ~2.4ns/descriptor aggregate, the
hard wall of this kernel.
"""

import numpy as np
import ml_dtypes

N = 100000
NPAD = 100352          # 784 * 128
D = 128
NCORES = 8
RPC = NPAD // NCORES   # 12544 rows per core
NT = RPC // 128        # 98 row tiles per core
CHUNK = 32768          # int16-index col chunks
NCH = 4
CALL = 1024            # descriptors per SWDGE gather call
GBUFS = 14             # gather call buffers in flight
PBUFS = 24             # P block buffers in flight

_cache = {}

bf16 = ml_dtypes.bfloat16


def _prep(input, adj_rows, adj_cols, adj_vals, weight, linear_weight, bias):
    f32 = np.float32

    lw = np.asarray(linear_weight, np.float64)
    e = np.exp(lw - lw.max())
    mix = (e / e.sum()).astype(f32)
    mix0 = float(mix[0])

    rows = np.asarray(adj_rows).reshape(-1)
    cols = np.asarray(adj_cols).reshape(-1)
    sval = (np.asarray(adj_vals, f32) * mix[1:, None]).reshape(-1)

    core = rows // RPC

    per_core = []
    counts = np.zeros((NCORES, NCH * NT), np.int64)
    for k in range(NCORES):
        m = core == k
        r = (rows[m] - k * RPC).astype(np.int32)
        c = cols[m].astype(np.int32)
        v = sval[m]
        seg = (c >> 15) * NT + (r >> 7)          # chunk-major segments
        order = np.argsort(seg, kind="stable")
        seg = seg[order]
        counts[k] = np.bincount(seg, minlength=NCH * NT)
        per_core.append((r[order], c[order], v[order], seg))

    B = np.maximum(np.ceil(counts.max(axis=0) / 128).astype(np.int64), 1)
    seg_start = np.concatenate([[0], np.cumsum(B * 128)])
    epad = int(seg_start[-1])
    nblk = epad // 128

    xin = np.zeros((NPAD, D), bf16)
    xin[:N] = np.asarray(input, f32).astype(bf16)
    xlocT_f = np.zeros((D, NPAD), f32)
    xlocT_f[:, :N] = np.asarray(input, f32).T

    wmat = np.asarray(weight, f32).astype(bf16)
    bias_b = np.asarray(bias, f32).astype(bf16)[None, :]
    ones_b = np.ones((1, D), bf16)
    ident = np.eye(D, dtype=bf16)
    iota = np.broadcast_to(np.arange(D, dtype=f32).astype(bf16), (D, D)).copy()

    in_maps = []
    for k in range(NCORES):
        r, c, v, seg = per_core[k]
        ncnt = counts[k]
        within = np.arange(len(r)) - np.repeat(
            np.concatenate([[0], np.cumsum(ncnt)[:-1]]), ncnt
        )
        dest = seg_start[seg] + within

        cpad = np.zeros(epad, np.int32)
        spad = np.full(epad, -1.0, f32)
        vpad = np.zeros(epad, f32)
        cpad[dest] = c & 32767
        spad[dest] = (r & 127).astype(f32)
        vpad[dest] = v

        gidx16 = cpad.reshape(-1, 16).T.astype(np.int16)
        gidx = np.broadcast_to(gidx16, (8, 16, epad // 16)).reshape(128, epad // 16).copy()

        in_maps.append({
            "xin": xin,
            "gidx": gidx,
            "slot": spad.reshape(nblk, 128).T.astype(bf16).copy(),  # [128, nblk]
            "sval": vpad.reshape(nblk, 128).T.astype(bf16).copy(),
            "xlocT": np.ascontiguousarray(
                xlocT_f[:, k * RPC:(k + 1) * RPC].astype(bf16)
            ),
            "wmat": wmat,
            "biasb": bias_b,
            "onesb": ones_b,
            "ident": ident,
            "iota": iota,
        })
    return in_maps, B.reshape(NCH, NT), mix0


def _build(B, mix0):
    import concourse.bass as bass
    import concourse.bacc as bacc
    import concourse.mybir as mybir
    import concourse.tile as tile

    dt = mybir.dt
    alu = mybir.AluOpType
    nblk = int(B.sum())
    epad = nblk * 128

    nc = bacc.Bacc(None, num_swdge_queues=4)
    xin_d = nc.declare_dram_parameter("xin", [NPAD, D], dt.bfloat16, isOutput=False)
    gidx_d = nc.declare_dram_parameter("gidx", [128, epad // 16], dt.int16, isOutput=False)
    slot_d = nc.declare_dram_parameter("slot", [128, nblk], dt.bfloat16, isOutput=False)
    sval_d = nc.declare_dram_parameter("sval", [128, nblk], dt.bfloat16, isOutput=False)
    xlocT_d = nc.declare_dram_parameter("xlocT", [128, RPC], dt.bfloat16, isOutput=False)
    wmat_d = nc.declare_dram_parameter("wmat", [D, D], dt.bfloat16, isOutput=False)
    bias_d = nc.declare_dram_parameter("biasb", [1, D], dt.bfloat16, isOutput=False)
    ones_d = nc.declare_dram_parameter("onesb", [1, D], dt.bfloat16, isOutput=False)
    ident_d = nc.declare_dram_parameter("ident", [D, D], dt.bfloat16, isOutput=False)
    iota_d = nc.declare_dram_parameter("iota", [D, D], dt.bfloat16, isOutput=False)
    out_d = nc.declare_dram_parameter("out", [RPC, D], dt.float32, isOutput=True)
    rep_d = nc.declare_dram_parameter("rep", [RPC, D], dt.float32, isOutput=True)

    # static gather plan: calls merge blocks within one chunk, up to CALL idxs
    calls = []
    b0 = 0
    for c in range(NCH):
        cb = int(B[c].sum())
        q = 0
        while q < cb:
            nb = min(CALL // 128, cb - q)
            calls.append((c, b0 + q, nb))
            q += nb
        b0 += cb

    with tile.TileContext(nc) as tc:
        with (
            tc.tile_pool(name="const", bufs=1) as const,
            tc.tile_pool(name="adj", bufs=1) as adj,
            tc.tile_pool(name="racc", bufs=1) as racc,
            tc.tile_pool(name="gbuf", bufs=GBUFS) as gbuf,
            tc.tile_pool(name="pbuf", bufs=PBUFS) as pbuf,
            tc.tile_pool(name="rbuf", bufs=4) as rbuf,
            tc.tile_pool(name="ps_acc", bufs=4, space="PSUM") as ps_acc,
            tc.tile_pool(name="ps_out", bufs=2, space="PSUM") as ps_out,
            tc.tile_pool(name="ps_rep", bufs=2, space="PSUM") as ps_rep,
        ):
            wmat = const.tile([D, D], dt.bfloat16)
            biasb = const.tile([1, D], dt.bfloat16)
            onesb = const.tile([1, D], dt.bfloat16)
            ident = const.tile([D, D], dt.bfloat16)
            iota = const.tile([D, D], dt.bfloat16)
            xlocT = const.tile([128, RPC], dt.bfloat16)
            gidx = adj.tile([128, epad // 16], dt.int16)
            slot = adj.tile([128, nblk], dt.bfloat16)
            sval = adj.tile([128, nblk], dt.bfloat16)
            repT = racc.tile([128, NT * D], dt.float32)

            nc.sync.dma_start(wmat[:], wmat_d[:])
            nc.sync.dma_start(biasb[:], bias_d[:])
            nc.sync.dma_start(onesb[:], ones_d[:])
            nc.sync.dma_start(ident[:], ident_d[:])
            nc.sync.dma_start(iota[:], iota_d[:])
            nc.sync.dma_start(xlocT[:], xlocT_d[:])
            nc.sync.dma_start(gidx[:], gidx_d[:])
            nc.sync.dma_start(slot[:], slot_d[:])
            nc.sync.dma_start(sval[:], sval_d[:])

            # SWDGE gather calls (chunk-major, 1024 descriptors each)
            gts = {}
            for qn, (c, blk0, nb) in enumerate(calls):
                gt = gbuf.tile([128, nb * 128], dt.bfloat16, tag="gt")
                gt3 = gt[:].rearrange("p (b e) -> p b e", e=128)
                nc.gpsimd.dma_gather(
                    out_ap=gt3,
                    in_ap=xin_d[c * CHUNK:min((c + 1) * CHUNK, NPAD), :],
                    idxs_ap=gidx[:, blk0 * 8:(blk0 + nb) * 8],
                    num_idxs=nb * 128,
                    num_idxs_reg=nb * 128,
                    elem_size=D,
                    queue_num=qn % 4,
                )
                for j in range(nb):
                    gts[blk0 + j] = (gt3, j)

            # per-(chunk, tile): P blocks via one fused STT each + matmul chain
            g = 0
            for c in range(NCH):
                for t in range(NT):
                    bc = int(B[c, t])
                    acc = ps_acc.tile([D, D], dt.float32)
                    for j in range(bc):
                        gt3, gj = gts[g + j]
                        pt = pbuf.tile([D, D], dt.bfloat16, tag="p")
                        sv = sval[:, g + j:g + j + 1]
                        sval_bc = bass.AP(sv.tensor, sv.offset, [sv.ap[0], [0, D]])
                        nc.vector.scalar_tensor_tensor(
                            pt[:], iota[:], slot[:, g + j:g + j + 1],
                            sval_bc, alu.is_equal, alu.mult,
                        )
                        nc.tensor.matmul(
                            acc[:], gt3[:, gj, :], pt[:],
                            start=(j == 0), stop=(j == bc - 1),
                        )
                    g += bc
                    if c == 0:
                        nc.vector.tensor_copy(repT[:, t * D:(t + 1) * D], acc[:])
                    else:
                        nc.vector.tensor_add(
                            repT[:, t * D:(t + 1) * D],
                            repT[:, t * D:(t + 1) * D], acc[:],
                        )

            # epilogue per 128-row tile
            for t in range(NT):
                rbf = rbuf.tile([D, D], dt.bfloat16, tag="rbf")
                nc.vector.scalar_tensor_tensor(
                    rbf[:], xlocT[:, t * D:(t + 1) * D], mix0,
                    repT[:, t * D:(t + 1) * D], alu.mult, alu.add,
                )
                outp = ps_out.tile([D, D], dt.float32)
                nc.tensor.matmul(outp[:], rbf[:], wmat[:], start=True, stop=False)
                nc.tensor.matmul(outp[:], onesb[:], biasb[:], start=False, stop=True)
                repp = ps_rep.tile([D, D], dt.float32)
                nc.tensor.matmul(repp[:], rbf[:], ident[:], start=True, stop=True)
                outs = rbuf.tile([D, D], dt.float32, tag="outs")
                reps = rbuf.tile([D, D], dt.float32, tag="reps")
                nc.scalar.copy(outs[:], outp[:])
                nc.scalar.copy(reps[:], repp[:])
                nc.sync.dma_start(out_d[t * 128:(t + 1) * 128, :], outs[:])
                nc.sync.dma_start(rep_d[t * 128:(t + 1) * 128, :], reps[:])

    nc.compile()
    return nc


def kernel(**inputs):
    from concourse.bass_utils import run_bass_kernel_spmd

    in_maps, B, mix0 = _prep(**inputs)
    key = (tuple(B.reshape(-1)), round(mix0, 9))
    if key not in _cache:
        _cache.clear()
        _cache[key] = _build(B, mix0)
    nc = _cache[key]

    res = run_bass_kernel_spmd(nc, in_maps, list(range(NCORES)))
    out = np.concatenate([np.asarray(res.results[k]["out"]) for k in range(NCORES)])
    rep = np.concatenate([np.asarray(res.results[k]["rep"]) for k in range(NCORES)])
    return out[:N].astype(np.float32), rep[:N].astype(np.float32)

